# revision 16
# baseline (speedup 1.0000x reference)
"""Sparse (block-local) attention for B=2, Sq=2048, Sk=4096, D=1024, H=16.

Each query i attends to exactly keys {2i, 2i+1} (Sk/Sq == 2, no remainder),
so softmax is over 2 scores -> p1 = sigmoid((s1-s2)*scale), p2 = sigmoid((s2-s1)*scale).

Distribution: sequence-parallel over (batch, query-block). 8 cores, each takes
512 contiguous queries of one batch plus the matching 1024 contiguous keys.
No collectives needed; outputs are concatenated on the host.

Per-core device kernel (all matmuls bf16 with fp32 PSUM accumulation):
  Q  = x_s  @ Wq^T           row-major   [512, 1024]
  K  = c_perm @ Wk^T         row-major   [1024, 1024] (keys permuted even|odd)
  V  = c_perm @ Wv^T         row-major   [1024, 1024]
  s1/s2 row-wise dots on DVE (mul + grouped reduce per 64-dim head)
  p1/p2 on ACT (sigmoid), AV combine on DVE -> att [512, 1024]
  att^T via PE transposes, O = att @ Wo^T, DMA out.

Host side only reshapes/shards/casts: feature-major + partition-major tiled
layouts, keys permuted even|odd, cast to bf16, concatenate core outputs.

Engine budget: PE ~89us (the bottleneck), ACT does all projection-PSUM
copies so DVE is free to run attention as soon as its inputs land.
"""

import sys

for _p in ("/opt/trn_rl_repo",):
    if _p not in sys.path:
        sys.path.append(_p)

import numpy as np
import ml_dtypes

import concourse.bass as bass
import concourse.mybir as mybir
import concourse.tile as tile
from concourse import bacc
from concourse.bass_utils import run_bass_kernel_spmd
from concourse.masks import make_identity
from concourse.tile_rust import add_dep_helper

B, SQ, SK, D, H, HD = 2, 2048, 4096, 1024, 16, 64
N_CORES = 8
QL = B * SQ // N_CORES       # 512 queries per core
KL = 2 * QL                  # 1024 keys per core
QT = QL // 128               # 4 query tiles
NB = 512                     # psum bank width (fp32)
JT = D // NB                 # 2 output-column blocks per projection
DT = D // 128                # 8 feature tiles
SCALE = 1.0 / float(np.sqrt(HD))

FB = mybir.dt.bfloat16
F32 = mybir.dt.float32
BF = ml_dtypes.bfloat16


def _build(kd_tiles: int, with_bo: bool):
    """Build + finalize the per-core Bacc graph (SPMD: same graph on 8 cores)."""
    nc = bacc.Bacc("TRN2", target_bir_lowering=False)

    # All activation/weight inputs are host-arranged partition-major:
    # tensor[p, t, n] = logical[t*128 + p, n], so DMA descriptors are
    # per-partition contiguous. Inputs are merged by NEED ORDER and the
    # DMA chain is gated so each phase gets full HBM bandwidth:
    #   xw0 = x | wq[:, 0:512]   -> everything Q's jb0 groups need
    #   xw1 = wq[:, 512:1024]    -> Q's jb1 groups
    #   ck  = cT | wk            -> K projection
    #   wv, wo
    xw0 = nc.dram_tensor("xw0", [128, kd_tiles, QL + NB], FB,
                         kind="ExternalInput")
    xw1 = nc.dram_tensor("xw1", [128, kd_tiles, D - NB], FB,
                         kind="ExternalInput")
    ck = nc.dram_tensor("ck", [128, kd_tiles, KL + D], FB, kind="ExternalInput")
    wv = nc.dram_tensor("wv", [128, kd_tiles, D], FB, kind="ExternalInput")
    wo = nc.dram_tensor("wo", [128, DT, D], FB, kind="ExternalInput")
    bo = None
    if with_bo:
        bo = nc.dram_tensor("bo", [1, D], F32, kind="ExternalInput")
    out = nc.dram_tensor("out", [128, QT, D], F32, kind="ExternalOutput")

    with tile.TileContext(nc) as tc:
        with (
            tc.tile_pool(name="ins", bufs=1) as ins,
            tc.tile_pool(name="acts", bufs=1) as acts,
            tc.tile_pool(name="att", bufs=4) as att,
            tc.tile_pool(name="outs", bufs=4) as outs,
            tc.tile_pool(name="psum", bufs=6, space="PSUM") as psum,
            tc.tile_pool(name="psum_tr", bufs=2, space="PSUM") as psum_tr,
        ):
            # ---- inputs to SBUF (need-order chained DMAs) ------------------
            xw0_sb = ins.tile([128, kd_tiles, QL + NB], FB)
            xw1_sb = ins.tile([128, kd_tiles, D - NB], FB)
            ck_sb = ins.tile([128, kd_tiles, KL + D], FB)
            wv_sb = ins.tile([128, kd_tiles, D], FB)
            wo_sb = ins.tile([128, DT, D], FB)
            ident = ins.tile([128, 128], FB)

            d0 = nc.sync.dma_start(out=xw0_sb, in_=xw0[:])
            d1 = nc.sync.dma_start(out=xw1_sb, in_=xw1[:])
            d2 = nc.sync.dma_start(out=ck_sb, in_=ck[:])
            d3 = nc.sync.dma_start(out=wv_sb, in_=wv[:])
            d4 = nc.sync.dma_start(out=wo_sb, in_=wo[:])
            chain = [d0, d1, d2, d3, d4]
            bo_sb = None
            if with_bo:
                bo_sb = ins.tile([128, D], F32)
                chain.append(nc.sync.dma_start(out=bo_sb,
                                               in_=bo.to_broadcast((128, D))))
            for d_prev, d_next in zip(chain, chain[1:]):
                add_dep_helper(d_next.ins, d_prev.ins, sync=True)
            make_identity(nc, ident)

            # PE warm-up: dummy matmuls during the DMA head keep HAM busy so
            # the real stream starts at full clock, at zero wall-clock cost.
            warm = ins.tile([128, 128], FB)
            nc.vector.memset(warm, 1.0)
            wps = psum_tr.tile([128, 128], F32, tag="tr")
            for _ in range(72):
                nc.tensor.matmul(wps, lhsT=warm, rhs=warm, start=True, stop=True)

            def x_slice(kd, col0):
                return xw0_sb[:, kd, col0:col0 + 128]

            def wq_slice(kd, jb):
                if jb == 0:
                    return xw0_sb[:, kd, QL:QL + NB]
                return xw1_sb[:, kd, (jb - 1) * NB:jb * NB]

            def ct_slice(kd, col0):
                return ck_sb[:, kd, col0:col0 + 128]

            def wk_slice(kd, jb):
                return ck_sb[:, kd, KL + jb * NB:KL + (jb + 1) * NB]

            # ---- projections (psum copies all on ACT) ----------------------
            q_sb = acts.tile([128, QT, D], FB)           # Q row-major
            k_sb = acts.tile([128, 2 * QT, D], FB)       # rows 0..3 even, 4..7 odd
            v_sb = acts.tile([128, 2 * QT, D], FB)

            def mm_one(dst_tile, dst_idx, jb, lhs_fn, rhs_fn, nkd=kd_tiles):
                ps = psum.tile([128, NB], F32, tag="mm")
                for kd in range(nkd):
                    nc.tensor.matmul(
                        ps,
                        lhsT=lhs_fn(kd),
                        rhs=rhs_fn(kd, jb),
                        start=(kd == 0),
                        stop=(kd == nkd - 1),
                    )
                nc.scalar.copy(dst_tile[:, dst_idx, jb * NB:(jb + 1) * NB], ps)

            def mm_group(dst_tile, dst_idx, lhs_fn, rhs_fn):
                for jb in range(JT):
                    mm_one(dst_tile, dst_idx, jb, lhs_fn, rhs_fn)

            def proj_c(dst_tile, dst_idx, col0, w_fn):
                mm_group(
                    dst_tile, dst_idx,
                    lambda kd: ct_slice(kd, col0),
                    w_fn,
                )

            # attention state per query tile
            av_sb = acts.tile([128, QT, D], FB)          # p1*V_even + p2*V_odd

            def attention(qt):
                qv = q_sb[:, qt, :]
                ke = k_sb[:, qt, :]
                ko = k_sb[:, QT + qt, :]
                pe = att.tile([128, H, HD], FB, tag="prod")
                po = att.tile([128, H, HD], FB, tag="prod")
                nc.vector.tensor_mul(pe.rearrange("p h e -> p (h e)"), qv, ke)
                nc.vector.tensor_mul(po.rearrange("p h e -> p (h e)"), qv, ko)
                s1 = att.tile([128, H], F32, tag="s")
                s2 = att.tile([128, H], F32, tag="s")
                nc.vector.reduce_sum(out=s1, in_=pe, axis=mybir.AxisListType.X)
                nc.vector.reduce_sum(out=s2, in_=po, axis=mybir.AxisListType.X)
                d12 = att.tile([128, H], F32, tag="s")
                nc.vector.tensor_sub(d12, s1, s2)
                p1 = att.tile([128, H], F32, tag="s")
                p2 = att.tile([128, H], F32, tag="s")
                nc.scalar.activation(p1, d12, mybir.ActivationFunctionType.Sigmoid,
                                     scale=SCALE)
                nc.scalar.activation(p2, d12, mybir.ActivationFunctionType.Sigmoid,
                                     scale=-SCALE)
                t1 = att.tile([128, H, HD], F32, tag="prodf")
                t2 = att.tile([128, H, HD], F32, tag="prodf")
                ve = v_sb[:, qt, :].rearrange("p (h e) -> p h e", h=H)
                vo = v_sb[:, QT + qt, :].rearrange("p (h e) -> p h e", h=H)
                nc.vector.tensor_mul(t1, ve, p1.to_broadcast((128, H, HD)))
                nc.vector.tensor_mul(t2, vo, p2.to_broadcast((128, H, HD)))
                nc.vector.tensor_add(
                    av_sb[:, qt, :],
                    t1.rearrange("p h e -> p (h e)"),
                    t2.rearrange("p h e -> p (h e)"),
                )

            # Q first, jb-outer: the jb0 groups only need xw0 (the first DMA),
            # jb1 groups unblock when xw1 lands
            for jb in range(JT):
                for qt in range(QT):
                    mm_one(q_sb, qt, jb,
                           lambda kd, qt=qt: x_slice(kd, qt * 128), wq_slice)
            # K/V pairs per qt; attention(qt) emitted one qt later so its ACT
            # sigmoid never stalls the projection-copy stream
            for qt in range(QT):
                proj_c(k_sb, qt, qt * 128, wk_slice)
                proj_c(k_sb, QT + qt, QL + qt * 128, wk_slice)
                proj_c(v_sb, qt, qt * 128,
                       lambda kd, jb: wv_sb[:, kd, jb * NB:(jb + 1) * NB])
                proj_c(v_sb, QT + qt, QL + qt * 128,
                       lambda kd, jb: wv_sb[:, kd, jb * NB:(jb + 1) * NB])
                if qt >= 1:
                    attention(qt - 1)
            attention(QT - 1)

            # ---- transpose att -> attT (copies on ACT), O groups interleaved
            avT_sb = acts.tile([128, DT, QL], FB)        # att^T feature-major

            def transposes(qt):
                for db in range(DT):
                    tp = psum_tr.tile([128, 128], FB, tag="tr")
                    nc.tensor.transpose(tp, av_sb[:, qt, db * 128:(db + 1) * 128],
                                        ident)
                    nc.scalar.copy(avT_sb[:, db, qt * 128:(qt + 1) * 128], tp)

            def o_group(qt):
                pss = [psum.tile([128, NB], F32, tag="mm", name=f"psg{jb}") for jb in range(JT)]
                for kd in range(DT):
                    for jb in range(JT):
                        nc.tensor.matmul(
                            pss[jb],
                            lhsT=avT_sb[:, kd, qt * 128:(qt + 1) * 128],
                            rhs=wo_sb[:, kd, jb * NB:(jb + 1) * NB],
                            start=(kd == 0),
                            stop=(kd == DT - 1),
                        )
                for jb in range(JT):
                    o_t = outs.tile([128, NB], F32, tag="o")
                    if with_bo:
                        nc.vector.tensor_add(o_t, pss[jb],
                                             bo_sb[:, jb * NB:(jb + 1) * NB])
                    else:
                        nc.vector.tensor_copy(o_t, pss[jb])
                    nc.sync.dma_start(out=out[:, qt, jb * NB:(jb + 1) * NB],
                                      in_=o_t)

            # PE order: T0 T1 O0 T2 O1 T3 O2 O3 — keeps PE fed while ACT
            # copies each avT tile group
            transposes(0)
            transposes(1)
            o_group(0)
            transposes(2)
            o_group(1)
            transposes(3)
            o_group(2)
            o_group(3)

    nc.finalize()
    return nc


_GRAPH_CACHE = {}


def _get_graph(kd_tiles: int, with_bo: bool):
    key = (kd_tiles, with_bo)
    if key not in _GRAPH_CACHE:
        _GRAPH_CACHE[key] = _build(kd_tiles, with_bo)
    return _GRAPH_CACHE[key]


def _pmajor(a, kd_tiles):
    """[kd_tiles*128, n] -> [128, kd_tiles, n] partition-major, contiguous."""
    n = a.shape[1]
    return np.ascontiguousarray(
        a.reshape(kd_tiles, 128, n).transpose(1, 0, 2))


def _make_in_maps(x, c, Wq, bq, Wk, bk, Wv, bv, Wo, bo):
    x = np.asarray(x, np.float32)
    c = np.asarray(c, np.float32)
    has_bias = any(np.any(np.asarray(b)) for b in (bq, bk, bv))
    with_bo = bool(np.any(np.asarray(bo)))
    kd_tiles = DT + (1 if has_bias else 0)
    KD = kd_tiles * 128

    def aug_w(W, b):
        wT = np.asarray(W, np.float32).T          # [D, D] feature-major
        if has_bias:
            pad = np.zeros((KD - D, D), np.float32)
            pad[0, :] = np.asarray(b, np.float32)
            wT = np.concatenate([wT, pad], axis=0)
        return _pmajor(wT.astype(BF), kd_tiles)

    wq_h = aug_w(Wq, bq)
    wk_h = aug_w(Wk, bk)
    wv_h = aug_w(Wv, bv)
    wo_h = _pmajor(np.ascontiguousarray(np.asarray(Wo, np.float32).T).astype(BF),
                   DT)

    def aug_act(aT):
        if has_bias:
            pad = np.zeros((KD - D, aT.shape[1]), np.float32)
            pad[0, :] = 1.0
            aT = np.concatenate([aT, pad], axis=0)
        return _pmajor(aT.astype(BF), kd_tiles)

    in_maps = []
    for core in range(N_CORES):
        b = core // (N_CORES // B)
        q0 = (core % (N_CORES // B)) * QL
        k0 = 2 * q0
        xs = x[b, q0:q0 + QL]                      # [QL, D]
        cs = c[b, k0:k0 + KL]                      # [KL, D]
        cperm = np.concatenate([cs[0::2], cs[1::2]], axis=0)
        xT_h = aug_act(np.ascontiguousarray(xs.T))     # [128, kd, QL]
        cT_h = aug_act(np.ascontiguousarray(cperm.T))  # [128, kd, KL]
        m = {
            # merged, in DMA need-order (see _build)
            "xw0": np.ascontiguousarray(
                np.concatenate([xT_h, wq_h[:, :, 0:NB]], axis=2)),
            "xw1": np.ascontiguousarray(wq_h[:, :, NB:]),
            "ck": np.ascontiguousarray(np.concatenate([cT_h, wk_h], axis=2)),
            "wv": wv_h,
            "wo": wo_h,
        }
        if with_bo:
            m["bo"] = np.asarray(bo, np.float32).reshape(1, D)
        in_maps.append(m)
    return in_maps, kd_tiles, with_bo


def _gather(results):
    out = np.empty((B, SQ, D), np.float32)
    for core in range(N_CORES):
        b = core // (N_CORES // B)
        q0 = (core % (N_CORES // B)) * QL
        # device layout [128, QT, D] -> rows q = qt*128 + p
        arr = results[core]["out"]
        out[b, q0:q0 + QL] = arr.transpose(1, 0, 2).reshape(QL, D)
    return out


def kernel(**inputs) -> np.ndarray:
    in_maps, kd_tiles, with_bo = _make_in_maps(**inputs)
    nc = _get_graph(kd_tiles, with_bo)
    res = run_bass_kernel_spmd(nc, in_maps, core_ids=list(range(N_CORES)))
    return _gather(res.results)


def run_traced(**inputs):
    """Like kernel() but with neuron-profile tracing; returns (out, results)."""
    in_maps, kd_tiles, with_bo = _make_in_maps(**inputs)
    nc = _get_graph(kd_tiles, with_bo)
    res = run_bass_kernel_spmd(nc, in_maps, core_ids=list(range(N_CORES)),
                               trace=True)
    return _gather(res.results), res


# revision 18
# speedup vs baseline: 1.0814x; 1.0814x over previous
"""Sparse (block-local) attention for B=2, Sq=2048, Sk=4096, D=1024, H=16.

Each query i attends to exactly keys {2i, 2i+1} (Sk/Sq == 2, no remainder),
so softmax is over 2 scores -> p1 = sigmoid((s1-s2)*scale), p2 = sigmoid((s2-s1)*scale).

Distribution: sequence-parallel over (batch, query-block). 8 cores, each takes
512 contiguous queries of one batch plus the matching 1024 contiguous keys.
No collectives needed; outputs are concatenated on the host.

Per-core device kernel (all matmuls bf16 with fp32 PSUM accumulation):
  Q  = x_s  @ Wq^T           row-major   [512, 1024]
  K  = c_perm @ Wk^T         row-major   [1024, 1024] (keys permuted even|odd)
  V  = c_perm @ Wv^T         row-major   [1024, 1024]
  s1/s2 row-wise dots on DVE (mul + grouped reduce per 64-dim head)
  p1/p2 on ACT (sigmoid), AV combine on DVE -> att [512, 1024]
  att^T via PE transposes, O = att @ Wo^T, DMA out.

Host side only reshapes/shards/casts: feature-major + partition-major tiled
layouts, keys permuted even|odd, cast to bf16, concatenate core outputs.

Engine budget: PE ~89us (the bottleneck), ACT does all projection-PSUM
copies so DVE is free to run attention as soon as its inputs land.
"""

import sys

for _p in ("/opt/trn_rl_repo",):
    if _p not in sys.path:
        sys.path.append(_p)

import numpy as np
import ml_dtypes

import concourse.bass as bass
import concourse.mybir as mybir
import concourse.tile as tile
from concourse import bacc
from concourse.bass_utils import run_bass_kernel_spmd
from concourse.masks import make_identity
from concourse.tile_rust import add_dep_helper

B, SQ, SK, D, H, HD = 2, 2048, 4096, 1024, 16, 64
N_CORES = 8
QL = B * SQ // N_CORES       # 512 queries per core
KL = 2 * QL                  # 1024 keys per core
QT = QL // 128               # 4 query tiles
NB = 512                     # psum bank width (fp32)
JT = D // NB                 # 2 output-column blocks per projection
DT = D // 128                # 8 feature tiles
SCALE = 1.0 / float(np.sqrt(HD))

FB = mybir.dt.bfloat16
F32 = mybir.dt.float32
BF = ml_dtypes.bfloat16


def _build(kd_tiles: int, with_bo: bool):
    """Build + finalize the per-core Bacc graph (SPMD: same graph on 8 cores)."""
    nc = bacc.Bacc("TRN2", target_bir_lowering=False)

    # All activation/weight inputs are host-arranged partition-major:
    # tensor[p, t, n] = logical[t*128 + p, n], so DMA descriptors are
    # per-partition contiguous. Inputs are merged by NEED ORDER and the
    # DMA chain is gated so each phase gets full HBM bandwidth:
    #   xw0 = x | wq[:, 0:512]   -> everything Q's jb0 groups need
    #   xw1 = wq[:, 512:1024]    -> Q's jb1 groups
    #   ck  = cT | wk            -> K projection
    #   wv, wo
    xw0 = nc.dram_tensor("xw0", [128, kd_tiles, QL + NB], FB,
                         kind="ExternalInput")
    xw1 = nc.dram_tensor("xw1", [128, kd_tiles, D - NB], FB,
                         kind="ExternalInput")
    ck = nc.dram_tensor("ck", [128, kd_tiles, KL + D], FB, kind="ExternalInput")
    wv = nc.dram_tensor("wv", [128, kd_tiles, D], FB, kind="ExternalInput")
    wo = nc.dram_tensor("wo", [128, DT, D], FB, kind="ExternalInput")
    bo = None
    if with_bo:
        bo = nc.dram_tensor("bo", [1, D], F32, kind="ExternalInput")
    out = nc.dram_tensor("out", [128, QT, D], F32, kind="ExternalOutput")

    with tile.TileContext(nc) as tc:
        with (
            tc.tile_pool(name="ins", bufs=1) as ins,
            tc.tile_pool(name="acts", bufs=1) as acts,
            tc.tile_pool(name="att", bufs=4) as att,
            tc.tile_pool(name="outs", bufs=4) as outs,
            tc.tile_pool(name="psum", bufs=6, space="PSUM") as psum,
            tc.tile_pool(name="psum_tr", bufs=2, space="PSUM") as psum_tr,
        ):
            # ---- inputs to SBUF (need-order chained DMAs) ------------------
            xw0_sb = ins.tile([128, kd_tiles, QL + NB], FB)
            xw1_sb = ins.tile([128, kd_tiles, D - NB], FB)
            ck_sb = ins.tile([128, kd_tiles, KL + D], FB)
            wv_sb = ins.tile([128, kd_tiles, D], FB)
            wo_sb = ins.tile([128, DT, D], FB)
            ident = ins.tile([128, 128], FB)

            d0 = nc.sync.dma_start(out=xw0_sb, in_=xw0[:])
            d1 = nc.sync.dma_start(out=xw1_sb, in_=xw1[:])
            d2 = nc.sync.dma_start(out=ck_sb, in_=ck[:])
            d3 = nc.sync.dma_start(out=wv_sb, in_=wv[:])
            d4 = nc.sync.dma_start(out=wo_sb, in_=wo[:])
            # xw0 alone gets full bandwidth; then xw1 (small) and ck share;
            # wv after both; wo last
            add_dep_helper(d1.ins, d0.ins, sync=True)
            add_dep_helper(d2.ins, d0.ins, sync=True)
            add_dep_helper(d3.ins, d1.ins, sync=True)
            add_dep_helper(d3.ins, d2.ins, sync=True)
            add_dep_helper(d4.ins, d3.ins, sync=True)
            bo_sb = None
            if with_bo:
                bo_sb = ins.tile([128, D], F32)
                d5 = nc.sync.dma_start(out=bo_sb, in_=bo.to_broadcast((128, D)))
                add_dep_helper(d5.ins, d3.ins, sync=True)
            make_identity(nc, ident)

            # PE warm-up: dummy matmuls during the DMA head keep HAM busy so
            # the real stream starts at full clock, at zero wall-clock cost.
            warm = ins.tile([128, 128], FB)
            nc.vector.memset(warm, 1.0)
            wps = psum_tr.tile([128, 128], F32, tag="tr")
            for _ in range(76):
                nc.tensor.matmul(wps, lhsT=warm, rhs=warm, start=True, stop=True)

            def x_slice(kd, col0):
                return xw0_sb[:, kd, col0:col0 + 128]

            def wq_slice(kd, jb):
                if jb == 0:
                    return xw0_sb[:, kd, QL:QL + NB]
                return xw1_sb[:, kd, (jb - 1) * NB:jb * NB]

            def ct_slice(kd, col0):
                return ck_sb[:, kd, col0:col0 + 128]

            def wk_slice(kd, jb):
                return ck_sb[:, kd, KL + jb * NB:KL + (jb + 1) * NB]

            # ---- projections (psum copies all on ACT) ----------------------
            q_sb = acts.tile([128, QT, D], FB)           # Q row-major
            k_sb = acts.tile([128, 2 * QT, D], FB)       # rows 0..3 even, 4..7 odd
            v_sb = acts.tile([128, 2 * QT, D], FB)

            def mm_one(dst_tile, dst_idx, jb, lhs_fn, rhs_fn, nkd=kd_tiles):
                ps = psum.tile([128, NB], F32, tag="mm")
                for kd in range(nkd):
                    nc.tensor.matmul(
                        ps,
                        lhsT=lhs_fn(kd),
                        rhs=rhs_fn(kd, jb),
                        start=(kd == 0),
                        stop=(kd == nkd - 1),
                    )
                nc.scalar.copy(dst_tile[:, dst_idx, jb * NB:(jb + 1) * NB], ps)

            def mm_group(dst_tile, dst_idx, lhs_fn, rhs_fn):
                for jb in range(JT):
                    mm_one(dst_tile, dst_idx, jb, lhs_fn, rhs_fn)

            def proj_c(dst_tile, dst_idx, col0, w_fn):
                mm_group(
                    dst_tile, dst_idx,
                    lambda kd: ct_slice(kd, col0),
                    w_fn,
                )

            # attention state per query tile
            av_sb = acts.tile([128, QT, D], FB)          # p1*V_even + p2*V_odd

            def attention(qt):
                qv = q_sb[:, qt, :]
                ke = k_sb[:, qt, :]
                ko = k_sb[:, QT + qt, :]
                pe = att.tile([128, H, HD], FB, tag="prod")
                po = att.tile([128, H, HD], FB, tag="prod")
                nc.vector.tensor_mul(pe.rearrange("p h e -> p (h e)"), qv, ke)
                nc.vector.tensor_mul(po.rearrange("p h e -> p (h e)"), qv, ko)
                s1 = att.tile([128, H], F32, tag="s")
                s2 = att.tile([128, H], F32, tag="s")
                nc.vector.reduce_sum(out=s1, in_=pe, axis=mybir.AxisListType.X)
                nc.vector.reduce_sum(out=s2, in_=po, axis=mybir.AxisListType.X)
                d12 = att.tile([128, H], F32, tag="s")
                nc.vector.tensor_sub(d12, s1, s2)
                p1 = att.tile([128, H], F32, tag="s")
                p2 = att.tile([128, H], F32, tag="s")
                nc.scalar.activation(p1, d12, mybir.ActivationFunctionType.Sigmoid,
                                     scale=SCALE)
                nc.scalar.activation(p2, d12, mybir.ActivationFunctionType.Sigmoid,
                                     scale=-SCALE)
                t1 = att.tile([128, H, HD], F32, tag="prodf")
                t2 = att.tile([128, H, HD], F32, tag="prodf")
                ve = v_sb[:, qt, :].rearrange("p (h e) -> p h e", h=H)
                vo = v_sb[:, QT + qt, :].rearrange("p (h e) -> p h e", h=H)
                nc.vector.tensor_mul(t1, ve, p1.to_broadcast((128, H, HD)))
                nc.vector.tensor_mul(t2, vo, p2.to_broadcast((128, H, HD)))
                nc.vector.tensor_add(
                    av_sb[:, qt, :],
                    t1.rearrange("p h e -> p (h e)"),
                    t2.rearrange("p h e -> p (h e)"),
                )

            # Q first, jb-outer: the jb0 groups only need xw0 (the first DMA),
            # jb1 groups unblock when xw1 lands
            for jb in range(JT):
                for qt in range(QT):
                    mm_one(q_sb, qt, jb,
                           lambda kd, qt=qt: x_slice(kd, qt * 128), wq_slice)
            # K/V pairs per qt; attention(qt) emitted one qt later so its ACT
            # sigmoid never stalls the projection-copy stream
            for qt in range(QT):
                proj_c(k_sb, qt, qt * 128, wk_slice)
                proj_c(k_sb, QT + qt, QL + qt * 128, wk_slice)
                proj_c(v_sb, qt, qt * 128,
                       lambda kd, jb: wv_sb[:, kd, jb * NB:(jb + 1) * NB])
                proj_c(v_sb, QT + qt, QL + qt * 128,
                       lambda kd, jb: wv_sb[:, kd, jb * NB:(jb + 1) * NB])
                if qt >= 1:
                    attention(qt - 1)
            attention(QT - 1)

            # ---- transpose att -> attT (copies on ACT), O groups interleaved
            avT_sb = acts.tile([128, DT, QL], FB)        # att^T feature-major

            def transposes(qt):
                for db in range(DT):
                    tp = psum_tr.tile([128, 128], FB, tag="tr")
                    nc.tensor.transpose(tp, av_sb[:, qt, db * 128:(db + 1) * 128],
                                        ident)
                    nc.scalar.copy(avT_sb[:, db, qt * 128:(qt + 1) * 128], tp)

            def o_group(qt):
                pss = [psum.tile([128, NB], F32, tag="mm", name=f"psg{jb}") for jb in range(JT)]
                for kd in range(DT):
                    for jb in range(JT):
                        nc.tensor.matmul(
                            pss[jb],
                            lhsT=avT_sb[:, kd, qt * 128:(qt + 1) * 128],
                            rhs=wo_sb[:, kd, jb * NB:(jb + 1) * NB],
                            start=(kd == 0),
                            stop=(kd == DT - 1),
                        )
                for jb in range(JT):
                    o_t = outs.tile([128, NB], F32, tag="o")
                    if with_bo:
                        nc.vector.tensor_add(o_t, pss[jb],
                                             bo_sb[:, jb * NB:(jb + 1) * NB])
                    else:
                        nc.vector.tensor_copy(o_t, pss[jb])
                    nc.sync.dma_start(out=out[:, qt, jb * NB:(jb + 1) * NB],
                                      in_=o_t)

            # PE order: T0 T1 O0 T2 O1 T3 O2 O3 — keeps PE fed while ACT
            # copies each avT tile group
            transposes(0)
            transposes(1)
            o_group(0)
            transposes(2)
            o_group(1)
            transposes(3)
            o_group(2)
            o_group(3)

    nc.finalize()
    return nc


_GRAPH_CACHE = {}


def _get_graph(kd_tiles: int, with_bo: bool):
    key = (kd_tiles, with_bo)
    if key not in _GRAPH_CACHE:
        _GRAPH_CACHE[key] = _build(kd_tiles, with_bo)
    return _GRAPH_CACHE[key]


def _pmajor(a, kd_tiles):
    """[kd_tiles*128, n] -> [128, kd_tiles, n] partition-major, contiguous."""
    n = a.shape[1]
    return np.ascontiguousarray(
        a.reshape(kd_tiles, 128, n).transpose(1, 0, 2))


def _make_in_maps(x, c, Wq, bq, Wk, bk, Wv, bv, Wo, bo):
    x = np.asarray(x, np.float32)
    c = np.asarray(c, np.float32)
    has_bias = any(np.any(np.asarray(b)) for b in (bq, bk, bv))
    with_bo = bool(np.any(np.asarray(bo)))
    kd_tiles = DT + (1 if has_bias else 0)
    KD = kd_tiles * 128

    def aug_w(W, b):
        wT = np.asarray(W, np.float32).T          # [D, D] feature-major
        if has_bias:
            pad = np.zeros((KD - D, D), np.float32)
            pad[0, :] = np.asarray(b, np.float32)
            wT = np.concatenate([wT, pad], axis=0)
        return _pmajor(wT.astype(BF), kd_tiles)

    wq_h = aug_w(Wq, bq)
    wk_h = aug_w(Wk, bk)
    wv_h = aug_w(Wv, bv)
    wo_h = _pmajor(np.ascontiguousarray(np.asarray(Wo, np.float32).T).astype(BF),
                   DT)

    def aug_act(aT):
        if has_bias:
            pad = np.zeros((KD - D, aT.shape[1]), np.float32)
            pad[0, :] = 1.0
            aT = np.concatenate([aT, pad], axis=0)
        return _pmajor(aT.astype(BF), kd_tiles)

    in_maps = []
    for core in range(N_CORES):
        b = core // (N_CORES // B)
        q0 = (core % (N_CORES // B)) * QL
        k0 = 2 * q0
        xs = x[b, q0:q0 + QL]                      # [QL, D]
        cs = c[b, k0:k0 + KL]                      # [KL, D]
        cperm = np.concatenate([cs[0::2], cs[1::2]], axis=0)
        xT_h = aug_act(np.ascontiguousarray(xs.T))     # [128, kd, QL]
        cT_h = aug_act(np.ascontiguousarray(cperm.T))  # [128, kd, KL]
        m = {
            # merged, in DMA need-order (see _build)
            "xw0": np.ascontiguousarray(
                np.concatenate([xT_h, wq_h[:, :, 0:NB]], axis=2)),
            "xw1": np.ascontiguousarray(wq_h[:, :, NB:]),
            "ck": np.ascontiguousarray(np.concatenate([cT_h, wk_h], axis=2)),
            "wv": wv_h,
            "wo": wo_h,
        }
        if with_bo:
            m["bo"] = np.asarray(bo, np.float32).reshape(1, D)
        in_maps.append(m)
    return in_maps, kd_tiles, with_bo


def _gather(results):
    out = np.empty((B, SQ, D), np.float32)
    for core in range(N_CORES):
        b = core // (N_CORES // B)
        q0 = (core % (N_CORES // B)) * QL
        # device layout [128, QT, D] -> rows q = qt*128 + p
        arr = results[core]["out"]
        out[b, q0:q0 + QL] = arr.transpose(1, 0, 2).reshape(QL, D)
    return out


def kernel(**inputs) -> np.ndarray:
    in_maps, kd_tiles, with_bo = _make_in_maps(**inputs)
    nc = _get_graph(kd_tiles, with_bo)
    res = run_bass_kernel_spmd(nc, in_maps, core_ids=list(range(N_CORES)))
    return _gather(res.results)


def run_traced(**inputs):
    """Like kernel() but with neuron-profile tracing; returns (out, results)."""
    in_maps, kd_tiles, with_bo = _make_in_maps(**inputs)
    nc = _get_graph(kd_tiles, with_bo)
    res = run_bass_kernel_spmd(nc, in_maps, core_ids=list(range(N_CORES)),
                               trace=True)
    return _gather(res.results), res
